# revision 25
# baseline (speedup 1.0000x reference)
"""Trainium2 Bass kernel for the integrate-and-fire "Integrator" layer.

Semantics (matches the JAX reference exactly):
  input  x  [4, 200, 64, 64, 8] f32, split into 2 independent time chunks of 100.
  Per neuron (b,h,w,c) and per chunk: V += x_t; if V > 2.0: spike at t, V = 0.
  Output: spike raster, permuted to [B, T, W, C, H] = [4, 200, 64, 8, 64] f32.

Strategy: pure data parallel across 8 cores; 32768 independent sequences
per core as [128 partitions, 256 free]. The time recurrence is a custom
DVE op (5 ALU stages):
    W       = relu(state_{t-1}) + x_t
    state_t = W - 8.0 * (W > theta)
A spiking step yields W - 8 in (-6, -5] (sign encodes the spike; decoded
to V=0 by the next step's relu, bit-exact); a non-spiking step yields V.

One instruction runs a whole GROUP of chain steps: the state lives in a
persistent SBUF buffer og[P, 101, F] (row 0 = zeros); each group's
instruction uses out=og[:, t0+1 : t0+1+kg, :], in0=og[:, t0 : t0+kg, :] —
in0/out overlap shifted by one row. The DVE streams elements in AP order
and a row's SBUF write lands long before that row is read 256 elements
later, so the recurrence feeds through memory WITHIN the instruction
(validated bit-exact on HW). ~270ns/step vs ~425ns single-step.

The run is input-DMA-paced (13.1 MB/core at ~360 GB/s ~= 36 us); all
compute hides under the input stream except the last few steps. Hence:
  - group schedule [2,4, 10x8, 4,4,2,2,2]: small head groups start the
    chain early; small tail groups minimize post-last-arrival chain work.
  - spike extraction for groups 0-9 runs on the Scalar engine as a SINGLE
    activation pass: uint8(Sigmoid(state * -1e30)) — spikes (state <= -5)
    saturate to exactly 1.0; state in [0, 2.2] gives <= 0.5 which the
    uint8 cast rounds to 0 (validated exact on HW, incl. +-0 and 2^-24).
  - the tail (steps 86-99) binarizes with ONE DVE tensor_scalar is_lt
    (2x perf mode) + ONE out-DMA on the then-idle Sync engine (HWDGE).
  - input DMAs issue from Sync, bulk output DMAs from the Pool engine so
    a pending output never blocks an input's issue slot; uint8 output
    quarters the output bytes (3.3 MB/core).
"""

import numpy as np

from concourse import bacc, bass, mybir
from concourse import dve_ops as _dve_ops
from concourse.dve_spec import C0, C1, Spec, Src0, Src1, _has_src1, lower, relu
from concourse.dve_uop import DveOpSpec
from concourse.tile import TileContext
from concourse.bass_utils import run_bass_kernel_spmd

_THETA = 2.0
_KBIG = 8.0   # spike marker subtracted from W; any K > theta + 1 works
_T = 100  # chunk length (time steps per independent sequence)
_P = 128  # SBUF partitions
_F = 256  # sequences per partition per core (128*256 = 32768 per core)
_NC = 8

_GROUPS = [2, 4] + [10] * 8 + [4, 4, 2, 2, 2]
_KMAX = max(_GROUPS)
_TAIL0 = 10  # groups >= this: chains back-to-back, one combined extraction
_PACK_GS = {0, 1, 2}  # groups pair-packed on DVE instead of scalar uint8:
# g0,g1 fit in the ~4us DVE idle gap before g2's input arrives (free);
# g2's op is absorbed by the chain's ~0.26us/group input-pacing slack

_B, _TT, _H, _W, _C = 4, 200, 64, 64, 8


def _if_step_ref(in0, in1, s0, s1, imm2):
    # DVE relu has max(NaN, 0) = 0 semantics; inputs here are never NaN.
    w = np.maximum(np.nan_to_num(in0.astype(np.float32), nan=0.0), 0.0) + in1.astype(
        np.float32
    )
    return (w - s1 * (w > s0).astype(np.float32)).astype(np.float32)


def _register_if_step_op():
    """Register the fused IF-step custom DVE op (documented extension point:
    dve_ops.OPS + _SUB_OPCODE_FOR_NAME + CUSTOM_DVE_SPECS). K rides the s1
    scalar slot (not imm2) so in1 may have 2 free dims (the STT-shape struct
    has no imm2 field). The uops sha is computed at runtime so there is
    nothing to pin manually."""
    name = "IF_STEP2_ANT"
    for op in _dve_ops.OPS:
        if op.name == name:
            return op
    w = relu(Src0) + Src1
    spec = Spec(body=w - C1 * (w > C0), reference=_if_step_ref)
    row = _dve_ops._CUSTOM_DVE_ROW_BASE + len(_dve_ops.OPS)
    assert row < 0x20
    _dve_ops._SUB_OPCODE_FOR_NAME[name] = row
    ver = "v3"  # TRN2
    uops = lower(spec, ver=ver)
    sha = DveOpSpec(name=name, opcode=row, uops=uops, rd1_en=_has_src1(spec)).sha(ver)
    op = _dve_ops.DveOp(name, spec, subdim=False, uops_sha={ver: sha})
    _dve_ops.OPS.append(op)
    _dve_ops.CUSTOM_DVE_SPECS[name] = spec
    return op


_IF_STEP = _register_if_step_op()


def _spike_pair_ref(in0, in1, s0, s1, imm2):
    return (
        (in0 < 0).astype(np.float32) + s0 * (in1 < 0).astype(np.float32)
    ).astype(np.float32)


def _register_spike_pair_op():
    """out = (Src0 < 0) + s0*(Src1 < 0): packs two steps' spikes per output
    element (s0=2 -> bits 0/1 of a uint8). Same DVE cost as a 2x-mode is_lt
    over both rows (custom ops run 1x but on half the output elements), and
    halves the tail output bytes on the critical path."""
    name = "SPIKE_PAIR_ANT"
    for op in _dve_ops.OPS:
        if op.name == name:
            return op
    from concourse.dve_spec import Zero
    spec = Spec(
        body=(Src0 < Zero) + C0 * (Src1 < Zero), reference=_spike_pair_ref
    )
    row = _dve_ops._CUSTOM_DVE_ROW_BASE + len(_dve_ops.OPS)
    assert row < 0x20
    _dve_ops._SUB_OPCODE_FOR_NAME[name] = row
    ver = "v3"
    uops = lower(spec, ver=ver)
    sha = DveOpSpec(name=name, opcode=row, uops=uops, rd1_en=_has_src1(spec)).sha(ver)
    op = _dve_ops.DveOp(name, spec, subdim=False, uops_sha={ver: sha})
    _dve_ops.OPS.append(op)
    _dve_ops.CUSTOM_DVE_SPECS[name] = spec
    return op


_SPIKE_PAIR = _register_spike_pair_op()


def _build():
    nc = bacc.Bacc("TRN2", target_bir_lowering=False, debug=False)
    x = nc.declare_dram_parameter("x", [_P, 128, _F], mybir.dt.float32, isOutput=False)
    s = nc.declare_dram_parameter("s", [_P, _T, _F], mybir.dt.uint8, isOutput=True)
    with TileContext(nc) as tc:
        with (
            tc.tile_pool(name="xin", bufs=6) as xpool,
            tc.tile_pool(name="sout", bufs=10) as spool,
            tc.tile_pool(name="tail", bufs=1) as tpool,
            tc.tile_pool(name="state", bufs=1) as stpool,
        ):
            # persistent state history: row r = state after step r (row 0 = 0)
            og = stpool.tile([_P, _T + 1, _F], mybir.dt.float32, tag="og")
            nc.vector.memset(og[:, 0, :], 0.0)
            tail_t0 = sum(_GROUPS[:_TAIL0])

            outs = []  # (t0e, kge, so) — output DMAs deferred past all inputs

            def emit_extract(t0e, kge):
                # 1-pass spike extraction on the Scalar engine
                so = spool.tile([_P, _KMAX, _F], mybir.dt.uint8, tag="s")
                nc.scalar.activation(
                    out=so[:, :kge, :], in_=og[:, t0e + 1:t0e + 1 + kge, :],
                    func=mybir.ActivationFunctionType.Sigmoid,
                    bias=0.0, scale=-1e30,
                )
                outs.append((t0e, kge, so))

            # extraction for group g is EMITTED after chain g+2: Tile's WAR
            # tracking on the shared og tile is coarse and its scheduling
            # order varies build to build, so a chain op can end up waiting
            # on a scalar READ of the preceding group's rows — the two-group
            # emission delay keeps even an adversarial schedule from pairing
            # a chain op with a scalar read less than ~2 groups behind it,
            # and the chain stays input-DMA-paced.
            pending = []
            t0 = 0
            for g, kg in enumerate(_GROUPS):
                xt = xpool.tile([_P, _KMAX, _F], mybir.dt.float32, tag="x")
                if g < 4:
                    # split early input DMAs into column halves: more
                    # descriptor streams in flight while the DMA subsystem
                    # ramps (every trace shows ~8 us of sub-rate stream at
                    # the start), so the input-paced chain starts sooner
                    nc.sync.dma_start(out=xt[:, :kg, :_F // 2],
                                      in_=x[:, t0:t0 + kg, :_F // 2])
                    nc.sync.dma_start(out=xt[:, :kg, _F // 2:],
                                      in_=x[:, t0:t0 + kg, _F // 2:])
                else:
                    nc.sync.dma_start(out=xt[:, :kg, :], in_=x[:, t0:t0 + kg, :])
                # whole group's recurrence in ONE instruction (see module doc)
                nc.vector._custom_dve(
                    _IF_STEP,
                    out=og[:, t0 + 1:t0 + 1 + kg, :],
                    in0=og[:, t0:t0 + kg, :],
                    in1=xt[:, :kg, :],
                    s0=_THETA,
                    s1=_KBIG,
                    imm2=0.0,
                )
                if g in _PACK_GS:
                    # pair-pack this group's spikes on DVE in its input-pacing
                    # slack (intra-engine, no WAR semaphores); halves its
                    # output bytes on the bytes-bound drain
                    sop = tpool.tile([_P, kg // 2, _F], mybir.dt.uint8,
                                     tag=f"spk{g}")
                    nc.vector._custom_dve(
                        _SPIKE_PAIR,
                        out=sop[:],
                        in0=og[:, t0 + 1:t0 + kg:2, :],
                        in1=og[:, t0 + 2:t0 + kg + 1:2, :],
                        s0=2.0, s1=0.0, imm2=0.0,
                    )
                    outs.append((t0, kg // 2, sop))
                if len(pending) >= 1:
                    emit_extract(*pending.pop(0))
                if g < _TAIL0 and g not in _PACK_GS:
                    pending.append((t0, kg))
                t0 += kg
            for p in pending:
                emit_extract(*p)
            pending = []
            # combined tail extraction, PAIR-PACKED: one custom op emits
            # (even<0) + 2*(odd<0) for steps tail_t0..T-1 -> kp bytes; the
            # host unpacks bits 0/1. Halves the tail bytes on the critical
            # path at the same DVE cost as the plain is_lt.
            kp = (_T - tail_t0) // 2
            sot = tpool.tile([_P, kp, _F], mybir.dt.uint8, tag="stail")
            nc.vector._custom_dve(
                _SPIKE_PAIR,
                out=sot[:],
                in0=og[:, tail_t0 + 1:_T:2, :],
                in1=og[:, tail_t0 + 2:_T + 1:2, :],
                s0=2.0, s1=0.0, imm2=0.0,
            )
            # ALL output DMAs issue on Sync AFTER every input DMA: the HW DMA
            # engines round-robin the active rings, so any output packets in
            # flight during the input phase steal input bandwidth at
            # schedule-dependent moments (the source of 57us-vs-65us
            # run-to-run bimodality). Deferring all output traffic past the
            # input stream keeps the input at full rate deterministically;
            # output data (ready long before) then drains in ~9us while the
            # tail computes. so tiles are never recycled (bufs=10) so the
            # scalar engine never waits on an output DMA.
            for t0e, kge, so in outs:
                nc.sync.dma_start(out=s[:, t0e:t0e + kge, :], in_=so[:, :kge, :])
            nc.sync.dma_start(out=s[:, tail_t0:tail_t0 + kp, :], in_=sot[:])
    return nc


def _shard(x):
    # [B, 200, H, W, C] -> per-core [128, 100, 256] f32, sequence-major
    xr = (
        x.reshape(_B, 2, _T, _H, _W, _C)
        .transpose(0, 1, 3, 4, 5, 2)  # [b, chunk, h, w, c, t]
        .reshape(-1, _T)              # [262144, 100]
    )
    per_core = xr.reshape(_NC, _P, _F, _T).transpose(0, 1, 3, 2)  # [8,128,100,256]
    # pad the time dim to 128 rows: power-of-two 128 KB partition stride in
    # DRAM (probe for HBM-bank sensitivity of the input stream)
    padded = np.zeros((_NC, _P, 128, _F), np.float32)
    padded[:, :, :_T, :] = per_core
    return [np.ascontiguousarray(padded[c]) for c in range(_NC)]


def _unshard(core_outs):
    # list of [128, 100, 256] (uint8) -> [B, T, W, C, H] f32
    raw = np.stack([np.asarray(o) for o in core_outs])  # uint8
    # unpack the pair-packed tail: byte v at row 86+i holds steps 86+2i
    # (bit 0) and 87+2i (bit 1)
    tail_t0 = sum(_GROUPS[:_TAIL0])
    kp = (_T - tail_t0) // 2
    packed = raw[:, :, tail_t0:tail_t0 + kp, :]
    full = raw.copy()
    full[:, :, tail_t0:_T:2, :] = packed & 1
    full[:, :, tail_t0 + 1:_T:2, :] = packed >> 1
    for gp in sorted(_PACK_GS):
        g0 = sum(_GROUPS[:gp])
        kgp = _GROUPS[gp]
        pg = raw[:, :, g0:g0 + kgp // 2, :]
        full[:, :, g0:g0 + kgp:2, :] = pg & 1
        full[:, :, g0 + 1:g0 + kgp:2, :] = pg >> 1
    sp = full.astype(np.float32)
    sp = sp.transpose(0, 1, 3, 2).reshape(_B, 2, _H, _W, _C, _T)  # [b,k,h,w,c,t]
    out = sp.transpose(0, 1, 5, 3, 4, 2).reshape(_B, _TT, _W, _C, _H)
    return np.ascontiguousarray(out)


def _run(x, trace=False):
    nc = _build()
    nc.finalize()  # run Bacc passes (multi-wait splitting etc.); PJRT path skips it
    in_maps = [{"x": xc} for xc in _shard(np.asarray(x, dtype=np.float32))]
    res = run_bass_kernel_spmd(nc, in_maps, core_ids=list(range(_NC)), trace=trace)
    out = _unshard([r["s"] for r in res.results])
    return out, res


def kernel(inputs):
    out, _ = _run(inputs, trace=False)
    return out



# revision 26
# speedup vs baseline: 1.0194x; 1.0194x over previous
"""Trainium2 Bass kernel for the integrate-and-fire "Integrator" layer.

Semantics (matches the JAX reference exactly):
  input  x  [4, 200, 64, 64, 8] f32, split into 2 independent time chunks of 100.
  Per neuron (b,h,w,c) and per chunk: V += x_t; if V > 2.0: spike at t, V = 0.
  Output: spike raster, permuted to [B, T, W, C, H] = [4, 200, 64, 8, 64] f32.

Strategy: pure data parallel across 8 cores; 32768 independent sequences
per core as [128 partitions, 256 free]. The time recurrence is a custom
DVE op (5 ALU stages):
    W       = relu(state_{t-1}) + x_t
    state_t = W - 8.0 * (W > theta)
A spiking step yields W - 8 in (-6, -5] (sign encodes the spike; decoded
to V=0 by the next step's relu, bit-exact); a non-spiking step yields V.

One instruction runs a whole GROUP of chain steps: the state lives in a
persistent SBUF buffer og[P, 101, F] (row 0 = zeros); each group's
instruction uses out=og[:, t0+1 : t0+1+kg, :], in0=og[:, t0 : t0+kg, :] —
in0/out overlap shifted by one row. The DVE streams elements in AP order
and a row's SBUF write lands long before that row is read 256 elements
later, so the recurrence feeds through memory WITHIN the instruction
(validated bit-exact on HW). ~270ns/step vs ~425ns single-step.

The run is input-DMA-paced (13.1 MB/core at ~360 GB/s ~= 36 us); all
compute hides under the input stream except the last few steps. Hence:
  - group schedule [2,4, 10x8, 4,4,2,2,2]: small head groups start the
    chain early; small tail groups minimize post-last-arrival chain work.
  - spike extraction for groups 0-9 runs on the Scalar engine as a SINGLE
    activation pass: uint8(Sigmoid(state * -1e30)) — spikes (state <= -5)
    saturate to exactly 1.0; state in [0, 2.2] gives <= 0.5 which the
    uint8 cast rounds to 0 (validated exact on HW, incl. +-0 and 2^-24).
  - the tail (steps 86-99) binarizes with ONE DVE tensor_scalar is_lt
    (2x perf mode) + ONE out-DMA on the then-idle Sync engine (HWDGE).
  - input DMAs issue from Sync, bulk output DMAs from the Pool engine so
    a pending output never blocks an input's issue slot; uint8 output
    quarters the output bytes (3.3 MB/core).
"""

import numpy as np

from concourse import bacc, bass, mybir
from concourse import dve_ops as _dve_ops
from concourse.dve_spec import C0, C1, Spec, Src0, Src1, _has_src1, lower, relu
from concourse.dve_uop import DveOpSpec
from concourse.tile import TileContext
from concourse.bass_utils import run_bass_kernel_spmd

_THETA = 2.0
_KBIG = 8.0   # spike marker subtracted from W; any K > theta + 1 works
_T = 100  # chunk length (time steps per independent sequence)
_P = 128  # SBUF partitions
_F = 256  # sequences per partition per core (128*256 = 32768 per core)
_NC = 8

_GROUPS = [2, 4] + [10] * 8 + [4, 4, 2, 2, 2]
_KMAX = max(_GROUPS)
_TAIL0 = 10  # groups >= this: chains back-to-back, one combined extraction
_PACK_GS = {0, 1, 2}  # groups pair-packed on DVE instead of scalar uint8:
# g0,g1 fit in the ~4us DVE idle gap before g2's input arrives (free);
# g2's op is absorbed by the chain's ~0.26us/group input-pacing slack

_B, _TT, _H, _W, _C = 4, 200, 64, 64, 8


def _if_step_ref(in0, in1, s0, s1, imm2):
    # DVE relu has max(NaN, 0) = 0 semantics; inputs here are never NaN.
    w = np.maximum(np.nan_to_num(in0.astype(np.float32), nan=0.0), 0.0) + in1.astype(
        np.float32
    )
    return (w - s1 * (w > s0).astype(np.float32)).astype(np.float32)


def _register_if_step_op():
    """Register the fused IF-step custom DVE op (documented extension point:
    dve_ops.OPS + _SUB_OPCODE_FOR_NAME + CUSTOM_DVE_SPECS). K rides the s1
    scalar slot (not imm2) so in1 may have 2 free dims (the STT-shape struct
    has no imm2 field). The uops sha is computed at runtime so there is
    nothing to pin manually."""
    name = "IF_STEP2_ANT"
    for op in _dve_ops.OPS:
        if op.name == name:
            return op
    w = relu(Src0) + Src1
    spec = Spec(body=w - C1 * (w > C0), reference=_if_step_ref)
    row = _dve_ops._CUSTOM_DVE_ROW_BASE + len(_dve_ops.OPS)
    assert row < 0x20
    _dve_ops._SUB_OPCODE_FOR_NAME[name] = row
    ver = "v3"  # TRN2
    uops = lower(spec, ver=ver)
    sha = DveOpSpec(name=name, opcode=row, uops=uops, rd1_en=_has_src1(spec)).sha(ver)
    op = _dve_ops.DveOp(name, spec, subdim=False, uops_sha={ver: sha})
    _dve_ops.OPS.append(op)
    _dve_ops.CUSTOM_DVE_SPECS[name] = spec
    return op


_IF_STEP = _register_if_step_op()


def _spike_pair_ref(in0, in1, s0, s1, imm2):
    return (
        (in0 < 0).astype(np.float32) + s0 * (in1 < 0).astype(np.float32)
    ).astype(np.float32)


def _register_spike_pair_op():
    """out = (Src0 < 0) + s0*(Src1 < 0): packs two steps' spikes per output
    element (s0=2 -> bits 0/1 of a uint8). Same DVE cost as a 2x-mode is_lt
    over both rows (custom ops run 1x but on half the output elements), and
    halves the tail output bytes on the critical path."""
    name = "SPIKE_PAIR_ANT"
    for op in _dve_ops.OPS:
        if op.name == name:
            return op
    from concourse.dve_spec import Zero
    spec = Spec(
        body=(Src0 < Zero) + C0 * (Src1 < Zero), reference=_spike_pair_ref
    )
    row = _dve_ops._CUSTOM_DVE_ROW_BASE + len(_dve_ops.OPS)
    assert row < 0x20
    _dve_ops._SUB_OPCODE_FOR_NAME[name] = row
    ver = "v3"
    uops = lower(spec, ver=ver)
    sha = DveOpSpec(name=name, opcode=row, uops=uops, rd1_en=_has_src1(spec)).sha(ver)
    op = _dve_ops.DveOp(name, spec, subdim=False, uops_sha={ver: sha})
    _dve_ops.OPS.append(op)
    _dve_ops.CUSTOM_DVE_SPECS[name] = spec
    return op


_SPIKE_PAIR = _register_spike_pair_op()


def _build():
    nc = bacc.Bacc("TRN2", target_bir_lowering=False, debug=False)
    x = nc.declare_dram_parameter("x", [_P, 128, _F], mybir.dt.float32, isOutput=False)
    s = nc.declare_dram_parameter("s", [_P, _T, _F], mybir.dt.uint8, isOutput=True)
    with TileContext(nc) as tc:
        with (
            tc.tile_pool(name="xin", bufs=6) as xpool,
            tc.tile_pool(name="sout", bufs=10) as spool,
            tc.tile_pool(name="tail", bufs=1) as tpool,
            tc.tile_pool(name="state", bufs=1) as stpool,
        ):
            # persistent state history: row r = state after step r (row 0 = 0)
            og = stpool.tile([_P, _T + 1, _F], mybir.dt.float32, tag="og")
            nc.vector.memset(og[:, 0, :], 0.0)
            tail_t0 = sum(_GROUPS[:_TAIL0])

            outs = []  # (t0e, kge, so) — output DMAs deferred past all inputs

            def emit_extract(t0e, kge):
                # 1-pass spike extraction on the Scalar engine
                so = spool.tile([_P, _KMAX, _F], mybir.dt.uint8, tag="s")
                nc.scalar.activation(
                    out=so[:, :kge, :], in_=og[:, t0e + 1:t0e + 1 + kge, :],
                    func=mybir.ActivationFunctionType.Sigmoid,
                    bias=0.0, scale=-1e30,
                )
                outs.append((t0e, kge, so))

            # extraction for group g is EMITTED after chain g+2: Tile's WAR
            # tracking on the shared og tile is coarse and its scheduling
            # order varies build to build, so a chain op can end up waiting
            # on a scalar READ of the preceding group's rows — the two-group
            # emission delay keeps even an adversarial schedule from pairing
            # a chain op with a scalar read less than ~2 groups behind it,
            # and the chain stays input-DMA-paced.
            pending = []
            t0 = 0
            for g, kg in enumerate(_GROUPS):
                xt = xpool.tile([_P, _KMAX, _F], mybir.dt.float32, tag="x")
                nc.sync.dma_start(out=xt[:, :kg, :], in_=x[:, t0:t0 + kg, :])
                # whole group's recurrence in ONE instruction (see module doc)
                nc.vector._custom_dve(
                    _IF_STEP,
                    out=og[:, t0 + 1:t0 + 1 + kg, :],
                    in0=og[:, t0:t0 + kg, :],
                    in1=xt[:, :kg, :],
                    s0=_THETA,
                    s1=_KBIG,
                    imm2=0.0,
                )
                if g in _PACK_GS:
                    # pair-pack this group's spikes on DVE in its input-pacing
                    # slack (intra-engine, no WAR semaphores); halves its
                    # output bytes on the bytes-bound drain
                    sop = tpool.tile([_P, kg // 2, _F], mybir.dt.uint8,
                                     tag=f"spk{g}")
                    nc.vector._custom_dve(
                        _SPIKE_PAIR,
                        out=sop[:],
                        in0=og[:, t0 + 1:t0 + kg:2, :],
                        in1=og[:, t0 + 2:t0 + kg + 1:2, :],
                        s0=2.0, s1=0.0, imm2=0.0,
                    )
                    outs.append((t0, kg // 2, sop))
                if len(pending) >= 1:
                    emit_extract(*pending.pop(0))
                if g < _TAIL0 and g not in _PACK_GS:
                    pending.append((t0, kg))
                t0 += kg
            for p in pending:
                emit_extract(*p)
            pending = []
            # combined tail extraction, PAIR-PACKED: one custom op emits
            # (even<0) + 2*(odd<0) for steps tail_t0..T-1 -> kp bytes; the
            # host unpacks bits 0/1. Halves the tail bytes on the critical
            # path at the same DVE cost as the plain is_lt.
            kp = (_T - tail_t0) // 2
            sot = tpool.tile([_P, kp, _F], mybir.dt.uint8, tag="stail")
            nc.vector._custom_dve(
                _SPIKE_PAIR,
                out=sot[:],
                in0=og[:, tail_t0 + 1:_T:2, :],
                in1=og[:, tail_t0 + 2:_T + 1:2, :],
                s0=2.0, s1=0.0, imm2=0.0,
            )
            # ALL output DMAs issue on Sync AFTER every input DMA: the HW DMA
            # engines round-robin the active rings, so any output packets in
            # flight during the input phase steal input bandwidth at
            # schedule-dependent moments (the source of 57us-vs-65us
            # run-to-run bimodality). Deferring all output traffic past the
            # input stream keeps the input at full rate deterministically;
            # output data (ready long before) then drains in ~9us while the
            # tail computes. so tiles are never recycled (bufs=10) so the
            # scalar engine never waits on an output DMA.
            for t0e, kge, so in outs:
                nc.sync.dma_start(out=s[:, t0e:t0e + kge, :], in_=so[:, :kge, :])
            nc.sync.dma_start(out=s[:, tail_t0:tail_t0 + kp, :], in_=sot[:])
    return nc


def _shard(x):
    # [B, 200, H, W, C] -> per-core [128, 100, 256] f32, sequence-major
    xr = (
        x.reshape(_B, 2, _T, _H, _W, _C)
        .transpose(0, 1, 3, 4, 5, 2)  # [b, chunk, h, w, c, t]
        .reshape(-1, _T)              # [262144, 100]
    )
    per_core = xr.reshape(_NC, _P, _F, _T).transpose(0, 1, 3, 2)  # [8,128,100,256]
    # pad the time dim to 128 rows: power-of-two 128 KB partition stride in
    # DRAM (probe for HBM-bank sensitivity of the input stream)
    padded = np.zeros((_NC, _P, 128, _F), np.float32)
    padded[:, :, :_T, :] = per_core
    return [np.ascontiguousarray(padded[c]) for c in range(_NC)]


def _unshard(core_outs):
    # list of [128, 100, 256] (uint8) -> [B, T, W, C, H] f32
    raw = np.stack([np.asarray(o) for o in core_outs])  # uint8
    # unpack the pair-packed tail: byte v at row 86+i holds steps 86+2i
    # (bit 0) and 87+2i (bit 1)
    tail_t0 = sum(_GROUPS[:_TAIL0])
    kp = (_T - tail_t0) // 2
    packed = raw[:, :, tail_t0:tail_t0 + kp, :]
    full = raw.copy()
    full[:, :, tail_t0:_T:2, :] = packed & 1
    full[:, :, tail_t0 + 1:_T:2, :] = packed >> 1
    for gp in sorted(_PACK_GS):
        g0 = sum(_GROUPS[:gp])
        kgp = _GROUPS[gp]
        pg = raw[:, :, g0:g0 + kgp // 2, :]
        full[:, :, g0:g0 + kgp:2, :] = pg & 1
        full[:, :, g0 + 1:g0 + kgp:2, :] = pg >> 1
    sp = full.astype(np.float32)
    sp = sp.transpose(0, 1, 3, 2).reshape(_B, 2, _H, _W, _C, _T)  # [b,k,h,w,c,t]
    out = sp.transpose(0, 1, 5, 3, 4, 2).reshape(_B, _TT, _W, _C, _H)
    return np.ascontiguousarray(out)


def _run(x, trace=False):
    nc = _build()
    nc.finalize()  # run Bacc passes (multi-wait splitting etc.); PJRT path skips it
    in_maps = [{"x": xc} for xc in _shard(np.asarray(x, dtype=np.float32))]
    res = run_bass_kernel_spmd(nc, in_maps, core_ids=list(range(_NC)), trace=trace)
    out = _unshard([r["s"] for r in res.results])
    return out, res


def kernel(inputs):
    out, _ = _run(inputs, trace=False)
    return out

